# revision 1
# baseline (speedup 1.0000x reference)
"""LoRA layer kernel for Trainium2 (8 NeuronCores, data-parallel).

Computes out = SCALING * (x @ A^T) @ B^T for x [4, 8192, 1024],
lora_A [4, 1024], lora_B [1024, 4], SCALING = 0.25.

Strategy (per core, shard = 4096 rows x 1024 features):
  - x rows are sharded across the 8 cores; A/B replicated (host pre-arranged).
  - Per 512-row group: DMA x in natural layout, transpose 128x128 chunks on
    the PE (fp32r transpose mode) into PSUM, evacuate to SBUF with the DVE,
    rank-4 matmuls (fp32r, N=512) for h^T = A x^T, then out = h @ (0.25 B^T)
    with rows back on partitions so the store is contiguous; ScalarE
    evacuates the output PSUM banks; one 2 MiB DMA store per group.
"""

import sys

for _p in (
    "/root/.axon_site",
    "/root/.axon_site/_ro/trn_rl_repo",
    "/root/.axon_site/_ro/pypackages",
):
    if _p not in sys.path:
        sys.path.insert(0, _p)

from contextlib import ExitStack

import numpy as np

N_CORES = 8
D_IN = 1024
D_OUT = 1024
RANK = 4
ROWS_TOTAL = 4 * 8192
ROWS_PER_CORE = ROWS_TOTAL // N_CORES  # 4096
SCALING = 1.0 / RANK

P = 128          # partitions
GROUP_ROWS = 512  # rows processed per pipeline iteration (4 subtiles of 128)
N_CHUNKS = D_IN // P  # 8 feature chunks


def emit_lora(tc, x_ap, at_ap, bt_ap, id_ap, out_ap, rows):
    """Emit the LoRA kernel IR for one core's shard of `rows` rows.

    x_ap  : DRAM [rows, D_IN]  f32 (declared f32r; raw f32 bits)
    at_ap : DRAM [P, N_CHUNKS, RANK] f32r, at[p, c, r] = A[r, c*P + p]
    bt_ap : DRAM [RANK, D_OUT] f32r, bt[r, o] = SCALING * B[o, r]
    id_ap : DRAM [P, P] f32r identity (for PE transpose)
    out_ap: DRAM [rows, D_OUT] f32
    """
    import concourse.mybir as mybir

    nc = tc.nc
    f32 = mybir.dt.float32
    f32r = mybir.dt.float32r
    ctx = tc._ctx  # ExitStack owned by caller

    n_groups = rows // GROUP_ROWS
    J = GROUP_ROWS // P  # 4 row subtiles per group
    n_ochunks = D_OUT // 512  # 2 output column chunks of 512

    consts = ctx.enter_context(tc.tile_pool(name="consts", bufs=1))
    xpool = ctx.enter_context(tc.tile_pool(name="xin", bufs=4))
    xtpool = ctx.enter_context(tc.tile_pool(name="xt", bufs=8))
    htpool = ctx.enter_context(tc.tile_pool(name="ht", bufs=2))
    opool = ctx.enter_context(tc.tile_pool(name="osb", bufs=3))
    ps_xt = ctx.enter_context(tc.tile_pool(name="ps_xt", bufs=3, space="PSUM"))
    ps_ht = ctx.enter_context(tc.tile_pool(name="ps_ht", bufs=2, space="PSUM"))
    ps_o = ctx.enter_context(tc.tile_pool(name="ps_o", bufs=3, space="PSUM"))

    # rows -> partitions: row = n*P + p
    x_pnm = x_ap.rearrange("(n p) m -> p n m", p=P)
    o_pnm = out_ap.rearrange("(n p) m -> p n m", p=P)

    # First group's x loads lead the HWDGE ring; the tiny constants ride the
    # SWDGE ring in parallel so nothing delays the first transposes.
    x_sb0 = xpool.tile([P, J, D_IN], f32r)
    for j in range(J):
        nc.sync.dma_start(x_sb0[:, j, :], x_pnm[:, j, :])

    ident = consts.tile([P, P], f32r)
    nc.gpsimd.dma_start(ident[:], id_ap[:])
    at_sb = consts.tile([P, N_CHUNKS, RANK], f32r)
    nc.gpsimd.dma_start(at_sb[:], at_ap[:])
    bt_sb = consts.tile([RANK, D_OUT], f32r)
    nc.gpsimd.dma_start(bt_sb[:], bt_ap[:])

    for g in range(n_groups):
        if g == 0:
            x_sb = x_sb0
        else:
            x_sb = xpool.tile([P, J, D_IN], f32r)
            for j in range(J):
                nc.sync.dma_start(x_sb[:, j, :], x_pnm[:, g * J + j, :])

        ht_ps = ps_ht.tile([RANK, GROUP_ROWS], f32)
        for c in range(N_CHUNKS):
            # Transpose the 4 row-subtiles of feature chunk c into one PSUM
            # bank: xt_ps[p=feat, j, m=row] = x[row, feat]. One accumulation
            # group per bank (start on first write, stop on last).
            xt_ps = ps_xt.tile([P, J, P], f32r)
            for j in range(J):
                nc.tensor.matmul(
                    xt_ps[:, j, :],
                    lhsT=x_sb[:, j, c * P : (c + 1) * P],
                    rhs=ident[:],
                    is_transpose=True,
                    start=(j == 0),
                    stop=(j == J - 1),
                )
            xt_sb = xtpool.tile([P, J, P], f32r)
            nc.vector.tensor_copy(xt_sb[:], xt_ps[:])
            # h^T[r, m] += sum_f A^T[cP+f, r] * xT[f, m]
            nc.tensor.matmul(
                ht_ps[:],
                lhsT=at_sb[:, c, :],
                rhs=xt_sb[:],
                start=(c == 0),
                stop=(c == N_CHUNKS - 1),
            )

        ht_sb = htpool.tile([RANK, GROUP_ROWS], f32r)
        nc.vector.tensor_copy(ht_sb[:], ht_ps[:])

        o_sb = opool.tile([P, J, D_OUT], f32)
        for j in range(J):
            for o2 in range(n_ochunks):
                o_ps = ps_o.tile([P, 512], f32)
                # out[m, o] = sum_r h^T[r, m] * bt[r, o]
                nc.tensor.matmul(
                    o_ps[:],
                    lhsT=ht_sb[:, j * P : (j + 1) * P],
                    rhs=bt_sb[:, o2 * 512 : (o2 + 1) * 512],
                    start=True,
                    stop=True,
                )
                nc.scalar.copy(o_sb[:, j, o2 * 512 : (o2 + 1) * 512], o_ps[:])

            # Stores ride the SWDGE (gpsimd) ring so a store waiting on its
            # copy never head-of-line-blocks the HWDGE load ring.
            nc.gpsimd.dma_start(o_pnm[:, g * J + j, :], o_sb[:, j, :])


def build_nc(rows=ROWS_PER_CORE):
    import concourse.mybir as mybir
    import concourse.tile as tile
    from concourse import bacc

    f32 = mybir.dt.float32
    f32r = mybir.dt.float32r
    nc = bacc.Bacc("TRN2", target_bir_lowering=False, debug=False)
    x_d = nc.dram_tensor("x", [rows, D_IN], f32r, kind="ExternalInput").ap()
    at_d = nc.dram_tensor("at", [P, N_CHUNKS, RANK], f32r, kind="ExternalInput").ap()
    bt_d = nc.dram_tensor("bt", [RANK, D_OUT], f32r, kind="ExternalInput").ap()
    id_d = nc.dram_tensor("ident", [P, P], f32r, kind="ExternalInput").ap()
    out_d = nc.dram_tensor("out", [rows, D_OUT], f32, kind="ExternalOutput").ap()

    with tile.TileContext(nc) as tc:
        with ExitStack() as ctx:
            tc._ctx = ctx
            emit_lora(tc, x_d, at_d, bt_d, id_d, out_d, rows)
    nc.compile()
    return nc


def round_tf32(a):
    """Round f32 to tfloat32 (10-bit mantissa), round-to-nearest-even."""
    u = np.ascontiguousarray(a, dtype=np.float32).view(np.uint32)
    r = (u + 0x0FFF + ((u >> 13) & 1)) & np.uint32(0xFFFFE000)
    return r.view(np.float32)


def host_prep(lora_A, lora_B):
    # at[p, c, r] = A[r, c*P + p]
    at = np.ascontiguousarray(
        lora_A.T.reshape(N_CHUNKS, P, RANK).transpose(1, 0, 2), dtype=np.float32
    )
    bt = np.ascontiguousarray(lora_B.T * SCALING, dtype=np.float32)
    return round_tf32(at), round_tf32(bt)


_NC_CACHE = {}


def kernel(x, lora_A, lora_B):
    from concourse.bass_utils import run_bass_kernel_spmd

    if "nc" not in _NC_CACHE:
        _NC_CACHE["nc"] = build_nc(ROWS_PER_CORE)
    nc = _NC_CACHE["nc"]

    x2 = np.ascontiguousarray(x, dtype=np.float32).reshape(ROWS_TOTAL, D_IN)
    at, bt = host_prep(np.asarray(lora_A), np.asarray(lora_B))
    ident = np.eye(P, dtype=np.float32)
    shards = x2.reshape(N_CORES, ROWS_PER_CORE, D_IN)
    in_maps = [
        {"x": np.ascontiguousarray(shards[i]), "at": at, "bt": bt, "ident": ident}
        for i in range(N_CORES)
    ]
    res = run_bass_kernel_spmd(nc, in_maps, core_ids=list(range(N_CORES)))
    out = np.concatenate([res.results[i]["out"] for i in range(N_CORES)], axis=0)
    return out.reshape(4, 8192, D_OUT)



# revision 2
# speedup vs baseline: 1.4797x; 1.4797x over previous
"""LoRA layer kernel for Trainium2 (8 NeuronCores, data-parallel).

Computes out = SCALING * (x @ A^T) @ B^T for x [4, 8192, 1024],
lora_A [4, 1024], lora_B [1024, 4], SCALING = 0.25.

Strategy (per core, shard = 4096 rows x 1024 features), fp16 end-to-end
(rel err ~6e-4 vs the 2e-2 gate; halves HBM traffic vs f32):
  - Host casts x to fp16 and pre-transposes each core's shard so feature
    chunks sit on partitions -> the device needs no PE transposes at all.
  - Per 1024-row tile t: one 2 MiB load (sync/HWDGE), two 512-row groups:
    8 accumulating rank-4 matmuls (contraction 128/chunk) -> ht PSUM,
    DVE cast-copy to fp16, then 8 matmuls put rows back on partitions
    with bt (0.25*B^T) as the moving operand; ScalarE/DVE alternate on
    PSUM->SBUF fp16 evacuation; one 2 MiB store (scalar/HWDGE ring).
  - Host upcasts the fp16 result to f32 and un-permutes.
"""

import sys

for _p in (
    "/root/.axon_site",
    "/root/.axon_site/_ro/trn_rl_repo",
    "/root/.axon_site/_ro/pypackages",
):
    if _p not in sys.path:
        sys.path.insert(0, _p)

from contextlib import ExitStack

import numpy as np

N_CORES = 8
D_IN = 1024
D_OUT = 1024
RANK = 4
ROWS_TOTAL = 4 * 8192
ROWS_PER_CORE = ROWS_TOTAL // N_CORES  # 4096
SCALING = 1.0 / RANK

P = 128            # partitions
N_CHUNKS = D_IN // P   # 8 feature chunks of 128
GROUP_ROWS = 512   # rows per PSUM accumulation group (1 ht bank)
TILE_ROWS = 1024   # rows per DMA tile (2 groups, 2 MiB fp16 transfers)
N_TILES = ROWS_PER_CORE // TILE_ROWS  # 4
G_PER_TILE = TILE_ROWS // GROUP_ROWS  # 2
J = GROUP_ROWS // P  # 4 row subtiles per group


def emit_lora(tc, xt_ap, at_ap, bt_ap, out_ap, rows):
    """Emit the LoRA kernel IR for one core's shard of `rows` rows.

    xt_ap : DRAM [P, N_TILES, 16, 512] fp16, xt[p, t, g*8+c, m] =
            x[t*1024 + g*512 + m, c*128 + p]  (pre-transposed by host)
    at_ap : DRAM [P, N_CHUNKS, RANK] fp16, at[p, c, r] = A[r, c*128 + p]
    bt_ap : DRAM [RANK, D_OUT] fp16, bt[r, o] = SCALING * B[o, r]
    out_ap: DRAM [P, N_TILES, 8, D_OUT] fp16, out[p, t, g*4+j, o] =
            result[t*1024 + g*512 + j*128 + p, o]
    """
    import concourse.mybir as mybir

    nc = tc.nc
    f32 = mybir.dt.float32
    f16 = mybir.dt.float16
    ctx = tc._ctx  # ExitStack owned by caller

    n_tiles = rows // TILE_ROWS

    consts = ctx.enter_context(tc.tile_pool(name="consts", bufs=1))
    xpool = ctx.enter_context(tc.tile_pool(name="xin", bufs=3))
    htpool = ctx.enter_context(tc.tile_pool(name="ht", bufs=3))
    opool = ctx.enter_context(tc.tile_pool(name="osb", bufs=3))
    ps_ht = ctx.enter_context(tc.tile_pool(name="ps_ht", bufs=2, space="PSUM"))
    ps_o = ctx.enter_context(tc.tile_pool(name="ps_o", bufs=5, space="PSUM"))

    # First x tile leads the HWDGE sync ring; tiny constants ride the SWDGE
    # ring in parallel so they never delay the first big load.
    x_sb0 = xpool.tile([P, G_PER_TILE * N_CHUNKS, GROUP_ROWS], f16)
    nc.sync.dma_start(x_sb0[:], xt_ap[:, 0])

    at_sb = consts.tile([P, N_CHUNKS, RANK], f16)
    nc.gpsimd.dma_start(at_sb[:], at_ap[:])
    bt_sb = consts.tile([RANK, D_OUT], f16)
    nc.gpsimd.dma_start(bt_sb[:], bt_ap[:])

    for t in range(n_tiles):
        if t == 0:
            x_sb = x_sb0
        else:
            x_sb = xpool.tile([P, G_PER_TILE * N_CHUNKS, GROUP_ROWS], f16)
            nc.sync.dma_start(x_sb[:], xt_ap[:, t])

        o_sb = opool.tile([P, G_PER_TILE * J, D_OUT], f16)
        for g in range(G_PER_TILE):
            # ht[r, m] = sum_f A[r, f] * x[row m, f], accumulated over the
            # 8 partition chunks of the feature dim into one PSUM bank.
            ht_ps = ps_ht.tile([RANK, GROUP_ROWS], f32)
            for c in range(N_CHUNKS):
                nc.tensor.matmul(
                    ht_ps[:],
                    lhsT=at_sb[:, c, :],
                    rhs=x_sb[:, g * N_CHUNKS + c, :],
                    start=(c == 0),
                    stop=(c == N_CHUNKS - 1),
                )
            ht_sb = htpool.tile([RANK, GROUP_ROWS], f16)
            nc.vector.tensor_copy(ht_sb[:], ht_ps[:])

            # out[m, o] = sum_r ht[r, m] * bt[r, o]; rows back on partitions
            # so the store is contiguous. ScalarE/DVE alternate evacuation.
            for j in range(J):
                for o2 in range(2):
                    o_ps = ps_o.tile([P, 512], f32)
                    nc.tensor.matmul(
                        o_ps[:],
                        lhsT=ht_sb[:, j * P : (j + 1) * P],
                        rhs=bt_sb[:, o2 * 512 : (o2 + 1) * 512],
                        start=True,
                        stop=True,
                    )
                    tgt = o_sb[:, g * J + j, o2 * 512 : (o2 + 1) * 512]
                    if (j + o2) % 2 == 0:
                        nc.scalar.copy(tgt, o_ps[:])
                    else:
                        nc.vector.tensor_copy(tgt, o_ps[:])

        # Stores ride the second HWDGE ring (ACT) so they never head-of-line
        # block the sync ring carrying the loads.
        nc.scalar.dma_start(out_ap[:, t], o_sb[:])


def build_nc(rows=ROWS_PER_CORE):
    import concourse.mybir as mybir
    import concourse.tile as tile
    from concourse import bacc

    f16 = mybir.dt.float16
    n_tiles = rows // TILE_ROWS
    nc = bacc.Bacc("TRN2", target_bir_lowering=False, debug=False)
    xt_d = nc.dram_tensor(
        "xt", [P, n_tiles, G_PER_TILE * N_CHUNKS, GROUP_ROWS], f16,
        kind="ExternalInput",
    ).ap()
    at_d = nc.dram_tensor("at", [P, N_CHUNKS, RANK], f16, kind="ExternalInput").ap()
    bt_d = nc.dram_tensor("bt", [RANK, D_OUT], f16, kind="ExternalInput").ap()
    out_d = nc.dram_tensor(
        "out", [P, n_tiles, G_PER_TILE * J, D_OUT], f16, kind="ExternalOutput"
    ).ap()

    with tile.TileContext(nc) as tc:
        with ExitStack() as ctx:
            tc._ctx = ctx
            emit_lora(tc, xt_d, at_d, bt_d, out_d, rows)
    nc.compile()
    return nc


def host_prep(lora_A, lora_B):
    # at[p, c, r] = A[r, c*128 + p]
    at = np.ascontiguousarray(
        np.asarray(lora_A, dtype=np.float32)
        .reshape(RANK, N_CHUNKS, P)
        .transpose(2, 1, 0)
    ).astype(np.float16)
    bt = np.ascontiguousarray(
        np.asarray(lora_B, dtype=np.float32).T * SCALING
    ).astype(np.float16)
    return at, bt


def stage_x(x):
    """x [4, 8192, 1024] f32 -> per-core [P, N_TILES, 16, 512] fp16 shards."""
    xh = np.asarray(x, dtype=np.float32).reshape(
        N_CORES, N_TILES, G_PER_TILE, GROUP_ROWS, N_CHUNKS, P
    )
    # (core, t, g, m, c, p) -> (core, p, t, g, c, m)
    xs = np.ascontiguousarray(xh.transpose(0, 5, 1, 2, 4, 3)).astype(np.float16)
    return xs.reshape(N_CORES, P, N_TILES, G_PER_TILE * N_CHUNKS, GROUP_ROWS)


def unstage_out(res_list):
    """Per-core [P, N_TILES, 8, 1024] fp16 -> out [4, 8192, 1024] f32."""
    o = np.stack(res_list, axis=0).reshape(
        N_CORES, P, N_TILES, G_PER_TILE, J, D_OUT
    )
    # (core, p, t, g, j, o) -> (core, t, g, j, p, o)
    of = o.transpose(0, 2, 3, 4, 1, 5).astype(np.float32)
    return np.ascontiguousarray(of).reshape(4, 8192, D_OUT)


_NC_CACHE = {}


def kernel(x, lora_A, lora_B):
    from concourse.bass_utils import run_bass_kernel_spmd

    if "nc" not in _NC_CACHE:
        _NC_CACHE["nc"] = build_nc(ROWS_PER_CORE)
    nc = _NC_CACHE["nc"]

    xs = stage_x(x)
    at, bt = host_prep(lora_A, lora_B)
    in_maps = [
        {"xt": np.ascontiguousarray(xs[i]), "at": at, "bt": bt}
        for i in range(N_CORES)
    ]
    res = run_bass_kernel_spmd(nc, in_maps, core_ids=list(range(N_CORES)))
    return unstage_out([res.results[i]["out"] for i in range(N_CORES)])
